# revision 1
# baseline (speedup 1.0000x reference)
"""Trainium2 Bass kernel v2 for moe_routing (nn_CITADEL_15118284882566).

Math: per pair b,

    out[b] = sum_q qw[b,q] * relu( max_{l,kd} sims[b,q,l] * dw[b,l,kd]
                                   * [d_id[b,l,kd] == q_id[b,q]] )
             + dot(q_cls[b], d_cls[b])

Device strategy (data-parallel over B across 8 cores, 64 pairs/core,
16 groups of 4 pairs; partitions = 4 pairs x 32 queries):

1. DIFF2 = (d_id - q_id) + 2^-12 * dw via K-stacked fp16 matmuls
   (ids split hi/lo so every operand is fp16-exact).
2. ONE ACT pass: d2s = f16( |2^17 * DIFF2| ).
   match     -> 2^5 * dw  (<= 32, finite)
   non-match -> >= 2^17 * (1 - 2^-12) -> saturates to f16 +inf. EXACT
   one-sided split, so a MIN-tree can collapse kd before any fixup.
3. DVE min-tree over the 5 kd planes -> dmn = 2^5 * dw (match) or +inf
   (no match). (min == the unique match; multi-(q,l,kd) matches do not
   occur in this regime and would only flip max->min of two weights.)
4. NaN fixup on the collapsed [128,512] only: nz = dmn*0 (0 or NaN),
   msk = nz + dmn (2^5*dw or NaN); prd = sims * msk; NaN-skipping
   reduce_max over l; relu * qw.
5. Epilogue: one-hot matmul (scaled 2^-5) sums the 32 queries per
   pair; cls dots via elementwise mult + ones matmul.

PSUM layout per group: diff-A {kd0,kd1} 2 banks + diff-B {kd2..4} 3
banks + sims 2x1 banks = 7 of 8 banks, so diff-A of group g+1 overlaps
ACT-B of group g -> steady state is ACT-bound (~2.7us/group).
"""
import sys

sys.path.insert(0, "/opt/trn_rl_repo")

import numpy as np

B, LQ, LD, KQ, KD, D = 512, 32, 512, 1, 5, 128
NCORES = 8
BPC = B // NCORES          # 64 pairs per core
NB = 4                     # pairs per group
G = BPC // NB              # 16 groups
P = 128
JD = KD * LD               # 2560
KSTACK = 14
EPS = 2.0 ** -12
RSCALE = 2.0 ** 17         # ACT |x| input scale (non-match -> f16 inf)

_CACHED = {}

DEFAULT_OPTS = dict(
    dma_eng="gpsimd",        # "sync" (HWDGE) or "gpsimd" (SWDGE)
    io_bufs=6,
    big_bufs=4,
    warmup=0,              # N dummy 512-col matmuls to unthrottle PE HAM
    gp_nz=False,           # nz (NaN-gen TS) on GPSIMD (Pool TT is illegal,
                           # TS untested)
    flat_red=False,         # two flat [P,512] reduces instead of one 3D
    dummy_mm=0,            # extra repeated sims matmuls to warm the PE HAM
    solo_ends=True,        # per-group chains for first/last 2 groups
    rhx_sync=False,
    act_scopy=True,        # even-group sims copied PSUM->SBUF by ACT         # rhx (small, latency-critical) via sync HWDGE
)


def _build_module(**kw):
    opts = dict(DEFAULT_OPTS)
    opts.update(kw)
    import concourse.bacc as bacc
    import concourse.mybir as mybir
    from concourse import tile

    f16 = mybir.dt.float16
    f32 = mybir.dt.float32
    Alu = mybir.AluOpType
    Act = mybir.ActivationFunctionType

    nc = bacc.Bacc("TRN2", target_bir_lowering=False, debug=False)

    dq_d = nc.dram_tensor("dq", [G, D, NB * (LD + LQ)], f16, kind="ExternalInput")
    rhx_d = nc.dram_tensor("rhx", [G, KSTACK, JD + P], f16, kind="ExternalInput")
    epi_d = nc.dram_tensor("epi", [P, 2 * BPC + NB + 1 + G], f32, kind="ExternalInput")

    out_d = nc.dram_tensor("out", [NB, G + BPC], f32, kind="ExternalOutput")

    LA = 2 * LD            # diff-A cols (kd0,kd1)
    LB = 3 * LD            # diff-B cols (kd2,kd3,kd4)

    with tile.TileContext(nc) as tc:
        with (
            tc.tile_pool(name="sb_io", bufs=opts["io_bufs"]) as sb_io,
            tc.tile_pool(name="sb_big", bufs=opts["big_bufs"]) as sb_big,
            tc.tile_pool(name="sb_wk", bufs=3) as sb_wk,
            tc.tile_pool(name="sb_res", bufs=1) as sb_res,
            tc.tile_pool(name="ps_a", bufs=1, space="PSUM") as ps_a,
            tc.tile_pool(name="ps_b", bufs=1, space="PSUM") as ps_b,
            tc.tile_pool(name="ps_s", bufs=3, space="PSUM") as ps_s,
        ):
            res = sb_res.tile([P, G], f32)
            epi_t = sb_res.tile([P, 2 * BPC + NB + 1 + G], f32)
            nc.sync.dma_start(epi_t[:], epi_d[:])
            qw_all = epi_t[:, 2 * BPC + NB + 1:]

            # cls-dot path depends only on epi: run it up front so the tail
            # is just the tiny tok matmul + one combined output DMA
            qcT_t = epi_t[:, 0:BPC]
            dcT_t = epi_t[:, BPC:2 * BPC]
            e4s_t = epi_t[:, 2 * BPC:2 * BPC + NB]
            ones_t = epi_t[:, 2 * BPC + NB:2 * BPC + NB + 1]
            out_sb = sb_res.tile([NB, G + BPC], f32)
            cp = sb_res.tile([D, BPC], f32)
            nc.vector.tensor_tensor(cp[:], qcT_t, dcT_t, Alu.mult)
            cls_ps = ps_s.tile([1, BPC], f32, name="cls_ps", tag="spool")
            nc.tensor.matmul(cls_ps[:], ones_t, cp[:], start=True, stop=True)
            nc.vector.tensor_copy(out_sb[0:1, G:], cls_ps[:])

            dma = nc.gpsimd if opts["dma_eng"] == "gpsimd" else nc.sync

            if opts["warmup"]:
                # PE HAM unthrottles only after a ~3.4us contiguous busy
                # window; burn dummy matmuls (on the tiny epi tile, which is
                # DMA'd first) during the initial dT fill so the real
                # matmuls run at 2.4 GHz from group 0.
                ne = 2 * BPC + NB + 1 + G
                wu = ps_s.tile([P, ne], f32, name="wu", tag="spool")
                for _ in range(opts["warmup"]):
                    nc.tensor.matmul(wu[:], epi_t[:, 0:P], epi_t[:],
                                     start=True, stop=True)

            for g in range(G):
                rhx_t = sb_io.tile([KSTACK, JD + P], f16, name="rhx_t")
                dq_t = sb_io.tile([D, NB * (LD + LQ)], f16, name="dq_t")
                rhx_eng = nc.sync if (g == 0 or opts["rhx_sync"]) else dma
                rhx_eng.dma_start(rhx_t[:], rhx_d[g, :, :])
                (nc.sync if g == 0 else dma).dma_start(dq_t[:], dq_d[g, :, :])
                dT_t = dq_t[:, 0:NB * LD]
                qTx_t = dq_t[:, NB * LD:]

                lhsT = rhx_t[:, JD:JD + P]
                dA = ps_a.tile([P, LA], f32, name="dA")
                dB = ps_b.tile([P, LB], f32, name="dB")
                # one matmul output <= one PSUM bank (512 fp32 cols)
                for k in range(2):
                    nc.tensor.matmul(dA[:, k * LD:(k + 1) * LD], lhsT,
                                     rhx_t[:, k * LD:(k + 1) * LD],
                                     start=True, stop=True)
                for k in range(3):
                    nc.tensor.matmul(dB[:, k * LD:(k + 1) * LD], lhsT,
                                     rhx_t[:, (2 + k) * LD:(3 + k) * LD],
                                     start=True, stop=True)

                s_ps = ps_s.tile([P, LD], f32, name="s_ps", tag="spool")
                for b in range(NB):
                    nc.tensor.matmul(
                        s_ps[b * LQ:(b + 1) * LQ, :],
                        qTx_t[:, b * LQ:(b + 1) * LQ],
                        dT_t[:, b * LD:(b + 1) * LD],
                        start=True, stop=True,
                        tile_position=(0, b * LQ),
                    )
                if opts["solo_ends"] and g >= G - 2:
                    # first/last groups: per-group chain for shorter
                    # pipeline fill/drain
                    rA1 = sb_big.tile([P, LA], f16, name="rA1")
                    rB1 = sb_big.tile([P, LB], f16, name="rB1")
                    nc.scalar.activation(rA1[:], dA[:], Act.Abs,
                                         bias=0.0, scale=RSCALE)
                    nc.scalar.activation(rB1[:], dB[:], Act.Abs,
                                         bias=0.0, scale=RSCALE)
                    t01s = sb_wk.tile([P, LD], f16, name="t01s")
                    t23s = sb_wk.tile([P, LD], f16, name="t23s")
                    dmns = sb_wk.tile([P, LD], f16, name="dmns")
                    nc.vector.tensor_tensor(t01s[:], rA1[:, 0:LD],
                                            rA1[:, LD:], Alu.min)
                    nc.vector.tensor_tensor(t23s[:], rB1[:, 0:LD],
                                            rB1[:, LD:2 * LD], Alu.min)
                    nc.vector.tensor_tensor(t01s[:], t01s[:], t23s[:], Alu.min)
                    nc.vector.tensor_tensor(dmns[:], t01s[:],
                                            rB1[:, 2 * LD:], Alu.min)
                    nzs = sb_wk.tile([P, LD], f16, name="nzs")
                    msks = sb_wk.tile([P, LD], f16, name="msks")
                    nc.vector.tensor_scalar(nzs[:], dmns[:], 0.0, None, Alu.mult)
                    nc.vector.tensor_tensor(msks[:], nzs[:], dmns[:], Alu.add)
                    prds = sb_wk.tile([P, LD], f32, name="prds")
                    nc.vector.tensor_tensor(prds[:], s_ps[:], msks[:], Alu.mult)
                    mxs = sb_wk.tile([P, 1], f32, name="mxs")
                    nc.vector.reduce_max(mxs[:], prds[:],
                                         axis=mybir.AxisListType.X)
                    nc.vector.scalar_tensor_tensor(
                        res[:, g:g + 1], mxs[:], 0.0, qw_all[:, g:g + 1],
                        Alu.max, Alu.mult)
                    continue

                # batch DVE ops across pairs of groups: fill one half of the
                # 2-group-wide tiles per group, run the chain on odd g.
                half = g % 2
                if half == 0:
                    rA2 = sb_big.tile([P, 2 * LA], f16, name="rA2")
                    rB2 = sb_big.tile([P, 2 * LB], f16, name="rB2")
                    prd2 = sb_wk.tile([P, 2 * LD], f16, name="prd2")
                    sps_prev = s_ps
                nc.scalar.activation(rA2[:, half * LA:(half + 1) * LA], dA[:],
                                     Act.Abs, bias=0.0, scale=RSCALE)
                nc.scalar.activation(rB2[:, half * LB:(half + 1) * LB], dB[:],
                                     Act.Abs, bias=0.0, scale=RSCALE)
                if half == 0:
                    if opts["act_scopy"]:
                        scop = sb_wk.tile([P, LD], f16, name="scop")
                        nc.scalar.activation(scop[:], s_ps[:], Act.Copy,
                                             bias=0.0, scale=1.0)
                        scop_prev = scop
                    continue

                rA3 = rA2.rearrange("p (u c) -> p u c", u=2)
                rB3 = rB2.rearrange("p (u c) -> p u c", u=2)
                t01 = sb_wk.tile([P, 2 * LD], f16, name="t01")
                t23 = sb_wk.tile([P, 2 * LD], f16, name="t23")
                dmn = sb_wk.tile([P, 2 * LD], f16, name="dmn")
                t013 = t01.rearrange("p (u c) -> p u c", u=2)
                t233 = t23.rearrange("p (u c) -> p u c", u=2)
                dmn3 = dmn.rearrange("p (u c) -> p u c", u=2)
                nc.vector.tensor_tensor(t013[:], rA3[:, :, 0:LD],
                                        rA3[:, :, LD:2 * LD], Alu.min)
                nc.vector.tensor_tensor(t233[:], rB3[:, :, 0:LD],
                                        rB3[:, :, LD:2 * LD], Alu.min)
                nc.vector.tensor_tensor(t01[:], t01[:], t23[:], Alu.min)
                nc.vector.tensor_tensor(dmn3[:], t01.rearrange(
                    "p (u c) -> p u c", u=2)[:], rB3[:, :, 2 * LD:], Alu.min)

                nz = sb_wk.tile([P, 2 * LD], f16, name="nz")
                msk = sb_wk.tile([P, 2 * LD], f16, name="msk")
                enz = nc.gpsimd if opts["gp_nz"] else nc.vector
                enz.tensor_scalar(nz[:], dmn[:], 0.0, None, Alu.mult)
                nc.vector.tensor_tensor(msk[:], nz[:], dmn[:], Alu.add)
                if opts["act_scopy"]:
                    nc.vector.tensor_tensor(prd2[:, 0:LD], scop_prev[:],
                                            msk[:, 0:LD], Alu.mult)
                else:
                    nc.vector.tensor_tensor(prd2[:, 0:LD], sps_prev[:],
                                            msk[:, 0:LD], Alu.mult)
                nc.vector.tensor_tensor(prd2[:, LD:], s_ps[:],
                                        msk[:, LD:], Alu.mult)
                mx2 = sb_wk.tile([P, 2], f32, name="mx2")
                if opts["flat_red"]:
                    nc.vector.reduce_max(mx2[:, 0:1], prd2[:, 0:LD],
                                         axis=mybir.AxisListType.X)
                    nc.vector.reduce_max(mx2[:, 1:2], prd2[:, LD:],
                                         axis=mybir.AxisListType.X)
                else:
                    nc.vector.reduce_max(
                        mx2[:], prd2.rearrange("p (u c) -> p u c", u=2)[:],
                        axis=mybir.AxisListType.X)
                # res = relu(mx2) * qw in one fused op
                nc.vector.scalar_tensor_tensor(
                    res[:, g - 1:g + 1], mx2[:], 0.0, qw_all[:, g - 1:g + 1],
                    Alu.max, Alu.mult)

            # ---- epilogue: tok colsums + combined output DMA ----
            tok_ps = ps_s.tile([NB, G], f32, name="tok_ps", tag="spool")
            nc.tensor.matmul(tok_ps[:], e4s_t, res[:], start=True, stop=True)
            nc.vector.tensor_copy(out_sb[:, 0:G], tok_ps[:])
            nc.sync.dma_start(out_d[:], out_sb[:])

    nc.compile()
    return nc


def _prep_core_inputs(c, q_repr, q_w, q_ids, q_cls, d_repr, d_w, d_ids, d_cls):
    """Pure layout/packing for one core's 64 pairs."""
    s = slice(c * BPC, (c + 1) * BPC)
    qr = q_repr[s]          # [64, 32, 128] f32
    qw = q_w[s, :, 0]       # [64, 32]
    qi = q_ids[s, :, 0]     # [64, 32] int64
    qc = q_cls[s]           # [64, 128]
    dr = d_repr[s]          # [64, 512, 128]
    dw = d_w[s]             # [64, 512, 5]
    di = d_ids[s]           # [64, 512, 5]
    dc = d_cls[s]           # [64, 128]

    dT = np.ascontiguousarray(
        dr.reshape(G, NB, LD, D).transpose(0, 3, 1, 2).reshape(G, D, NB * LD)
    ).astype(np.float16)

    qTx = np.ascontiguousarray(
        qr.reshape(G, NB, LQ, D).transpose(0, 3, 1, 2).reshape(G, D, NB * LQ)
    ).astype(np.float16)
    qww = qw.reshape(G, NB * LQ)  # partition p = 32*b + q

    q_hi = (qi >> 8).astype(np.float32)
    q_lo = (qi & 255).astype(np.float32)
    d_hi = (di >> 8).astype(np.float32)
    d_lo = (di & 255).astype(np.float32)
    dw16 = dw.astype(np.float16).astype(np.float32)

    E = np.zeros((NB, P), np.float32)
    for b in range(NB):
        E[b, b * LQ:(b + 1) * LQ] = 1.0

    # rhx: [G, KSTACK, JD + P]: cols [0, JD) = rhs (kd-major), [JD, JD+P) = lhsT
    rhx = np.zeros((G, KSTACK, JD + P), np.float32)
    rhx[:, 0:4, :JD] = d_hi.reshape(G, NB, LD, KD).transpose(0, 1, 3, 2).reshape(G, NB, JD)
    rhx[:, 4:8, :JD] = d_lo.reshape(G, NB, LD, KD).transpose(0, 1, 3, 2).reshape(G, NB, JD)
    rhx[:, 8, :JD] = 256.0
    rhx[:, 9, :JD] = 1.0
    rhx[:, 10:14, :JD] = dw16.reshape(G, NB, LD, KD).transpose(0, 1, 3, 2).reshape(G, NB, JD)
    rhx[:, 0:4, JD:] = 256.0 * E
    rhx[:, 4:8, JD:] = E
    rhx[:, 8, JD:] = -q_hi.reshape(G, P)
    rhx[:, 9, JD:] = -q_lo.reshape(G, P)
    rhx[:, 10:14, JD:] = EPS * E

    epi = np.zeros((P, 2 * BPC + NB + 1 + G), np.float32)
    epi[:, 0:BPC] = qc.T
    epi[:, BPC:2 * BPC] = dc.T
    for b in range(NB):
        # undo the 2^5 the Abs-scale leaves on matched weights
        epi[b * LQ:(b + 1) * LQ, 2 * BPC + b] = 2.0 ** -5
    epi[:, 2 * BPC + NB] = 1.0
    epi[:, 2 * BPC + NB + 1:] = qww.T

    dq = np.concatenate([dT, qTx], axis=2)  # [G, D, NB*(LD+LQ)]
    return {
        "dq": dq,
        "rhx": rhx.astype(np.float16),
        "epi": epi,
    }


def kernel(q_expert_repr, q_expert_weights, q_expert_ids, q_cls_repr,
           d_expert_repr, d_expert_weights, d_expert_ids, d_cls_repr):
    from concourse.bass_utils import run_bass_kernel_spmd

    q_repr = np.asarray(q_expert_repr, np.float32)
    q_w = np.asarray(q_expert_weights, np.float32)
    q_ids = np.asarray(q_expert_ids, np.int64)
    q_cls = np.asarray(q_cls_repr, np.float32)
    d_repr = np.asarray(d_expert_repr, np.float32)
    d_w = np.asarray(d_expert_weights, np.float32)
    d_ids = np.asarray(d_expert_ids, np.int64)
    d_cls = np.asarray(d_cls_repr, np.float32)

    if "nc" not in _CACHED:
        _CACHED["nc"] = _build_module()
    nc = _CACHED["nc"]

    in_maps = [
        _prep_core_inputs(c, q_repr, q_w, q_ids, q_cls, d_repr, d_w, d_ids, d_cls)
        for c in range(NCORES)
    ]
    rr = run_bass_kernel_spmd(nc, in_maps, core_ids=list(range(NCORES)))

    out = np.zeros((B,), np.float32)
    for c in range(NCORES):
        r = rr.results[c]["out"]            # [NB, G + BPC]
        tok = r[:, 0:G]                     # [NB, G]
        cls = r[0, G:]                      # [BPC]
        out[c * BPC:(c + 1) * BPC] = tok.T.reshape(-1) + cls
    return out

